# revision 39
# baseline (speedup 1.0000x reference)
"""Self-contained Trainium2 Bass kernel for a 2-layer GAT (GATConv x2, PyG-style).

Contract: kernel(**inputs) takes the FULL inputs (x [N,128] f32, edge_index
[2,E] int, W1/att_src1/att_dst1/b1/W2/att_src2/att_dst2/b2) and returns the
FULL [N,128] f32 output, distributing work across 8 NeuronCores internally.

v2 design (destination-sharded graph parallel):
  - Nodes are split by orig id into two halves; half h is dealt across cores
    4h..4h+3 by (w0,w1)-sorted rank so all 8 cores see near-identical degree
    sequences per 128-dst block (small slot padding).
  - Per layer each core computes h = x @ W_ext only for its OWN 6272 nodes
    (49 x 128), packs 256-byte rows [124 bf16 feats | 4 fp8 feats | a_src f32]
    and one AllGather builds the full 50176-row gather table on every core.
  - Per dst block, source rows are fetched with dma_gather (int16 indices,
    two 25088-row windows) into [128 dst, t, 128] bf16 tiles; softmax runs
    per partition (no max-subtraction needed: |e| is small, exp is safe in
    f32); weighted sum = 1x multiply + bf16 pair-tree + small f32 reduce.
  - Layer-2 h rows are produced inline as layer-1 edge blocks complete, so
    only the AllGather itself sits between the two edge phases.
"""

import hashlib
from contextlib import ExitStack

import ml_dtypes
import numpy as np

# ---------------------------------------------------------------------------
# Configuration
# ---------------------------------------------------------------------------

F = 128              # feature dim (all of F_in, H, F_out)
CORES = 8
RB = 128             # bf16 elements per table row (256 B)
NBF = 124            # leading bf16 feature columns in a row
NF8 = 4              # fp8 feature columns (features 124..127)
ACOL = 63            # f32 column (within bitcast row) holding a_src
NEG_SLOPE = 0.2
MASK_NEG = -30000.0

import os as _os
NQUEUES = int(_os.environ.get("GAT_NQUEUES", "4"))
GCHUNK = int(_os.environ.get("GAT_GCHUNK", "8"))      # slot-cols per dma_gather
SCRATCH = int(_os.environ.get("GAT_SCRATCH", "16384"))
SINGLE_PACKET = _os.environ.get("GAT_SINGLE_PACKET", "1") == "1"
TREE_LEVELS = int(_os.environ.get("GAT_TREE_LEVELS", "2"))
PIPE = _os.environ.get("GAT_PIPE", "0") == "1"
ACT_EL = _os.environ.get("GAT_ACT_EL", "1") == "1"
ACT_OUT = _os.environ.get("GAT_ACT_OUT", "1") == "1"
EX8_MODE = _os.environ.get("GAT_EX8", "1") == "1"


class Cfg:
    def __init__(self, n_nodes, per_core_blocks):
        self.N = n_nodes
        self.NB = per_core_blocks              # blocks of 128 dsts per core
        self.PERP = per_core_blocks * 128      # padded positions per core
        self.PER = n_nodes // CORES            # real nodes per core
        assert self.PER * CORES == n_nodes
        assert self.PERP >= self.PER
        self.NPAD = self.PERP * CORES
        self.WIN = self.PERP * (CORES // 2)    # gather window boundary
        assert self.WIN - 1 < 32768, "window must fit int16"


FULL_CFG = Cfg(50000, 49)


# ---------------------------------------------------------------------------
# Host-side topology preprocessing (pure graph structure, no feature math)
# ---------------------------------------------------------------------------

def _snake_order(w0, w1):
    """Order dsts by (w0, w1) with alternating w1 direction per w0-run, so
    consecutive groups have near-constant (w0, w1)."""
    idx = np.lexsort((w1, w0))
    w0s = w0[idx]
    out = []
    i = 0
    flip = False
    while i < len(idx):
        j = i
        while j < len(idx) and w0s[j] == w0s[i]:
            j += 1
        run = idx[i:j]
        out.append(run[::-1] if flip else run)
        flip = not flip
        i = j
    return np.concatenate(out)


def build_topology(cfg, edge_index):
    """Returns the shared block structure + per-core gather metadata."""
    src = np.asarray(edge_index[0], dtype=np.int64)
    dst = np.asarray(edge_index[1], dtype=np.int64)
    N, PER, PERP, NB = cfg.N, cfg.PER, cfg.PERP, cfg.NB
    half = N // 2  # orig-id window boundary (cores 0-3 vs 4-7)

    w0all = np.bincount(dst[src < half], minlength=N)
    w1all = np.bincount(dst[src >= half], minlength=N)

    # deal each half's (w0,w1)-sorted ranks across its 4 cores so per-block
    # degree sequences line up across cores
    orig2pos = np.full(N, -1, dtype=np.int64)
    S0k = np.zeros((CORES, NB), np.int64)
    S1k = np.zeros((CORES, NB), np.int64)
    for g in (0, 1):
        ids = np.arange(half * g, half * (g + 1))
        order = _snake_order(w0all[ids], w1all[ids])
        nodes = ids[order]                      # rank -> node id
        r = np.arange(len(nodes))
        core = 4 * g + (r % 4)
        pos = r // 4
        orig2pos[nodes] = PERP * core + pos
        for c in range(4):
            k = 4 * g + c
            w0s = w0all[nodes[c::4]]
            w1s = w1all[nodes[c::4]]
            w0p = np.concatenate([w0s, np.zeros(PERP - len(w0s), np.int64)])
            w1p = np.concatenate([w1s, np.zeros(PERP - len(w1s), np.int64)])
            S0k[k] = w0p.reshape(NB, 128).max(1)
            S1k[k] = w1p.reshape(NB, 128).max(1)

    S0 = S0k.max(0)
    S1 = S1k.max(0)
    T = 1 + S0 + S1                             # self col + both windows

    pos2orig = np.full(cfg.NPAD, -1, dtype=np.int64)
    valid = orig2pos >= 0
    pos2orig[orig2pos[valid]] = np.nonzero(valid)[0]

    IA = int(8 * S0.sum())
    IB = int(8 * S1.sum())
    idxA = np.zeros((CORES, 128, max(IA, 16)), np.int16)
    idxB = np.zeros((CORES, 128, max(IB, 16)), np.int16)

    # padded slots gather the "poison" rows (each core's last own position,
    # which is padding on every core); its a_src is set to MASK_NEG by the
    # kernel so exp() kills the slot without a separate mask tensor.
    poisonA = int(_os.environ.get('GAT_POIA', cfg.WIN - 1))
    poisonB = int(_os.environ.get('GAT_POIB', cfg.NPAD - 1 - cfg.WIN))

    dpos = orig2pos[dst]
    spos = orig2pos[src]
    dcore = dpos // PERP
    for k in range(CORES):
        m = dcore == k
        pos_of_dst = dpos[m] - PERP * k
        sp = spos[m]
        isw0 = sp < cfg.WIN
        bucket0 = [[] for _ in range(PERP)]
        bucket1 = [[] for _ in range(PERP)]
        for p, s_p, w in zip(pos_of_dst, sp, isw0):
            (bucket0 if w else bucket1)[p].append(s_p)
        aoff = boff = 0
        for b in range(NB):
            s0, s1 = int(S0[b]), int(S1[b])
            flatA = np.full(128 * s0, poisonA, np.int64)
            flatB = np.full(128 * s1, poisonB, np.int64)
            for p in range(128):
                gidx = 128 * b + p
                for s_i, s_p in enumerate(bucket0[gidx]):
                    flatA[s_i * 128 + p] = s_p
                for s_i, s_p in enumerate(bucket1[gidx]):
                    flatB[s_i * 128 + p] = s_p - cfg.WIN
            # wrap indices into [128, 8*n] int16 (16-row layout, replicated)
            for nfl, arr, off, tgt in ((s0, flatA, aoff, idxA),
                                       (s1, flatB, boff, idxB)):
                if nfl == 0:
                    continue
                cols = 8 * nfl
                wrapped = arr.reshape(cols, 16).T.astype(np.int16)  # [16, cols]
                tgt[k, :, off:off + cols] = np.tile(wrapped, (8, 1))
            aoff += 8 * s0
            boff += 8 * s1

    stats = {
        "real_edges": int(len(src)) + N,
        "padded_edges": int(T.sum()) * 128 * CORES,
    }
    return {
        "S0": S0, "S1": S1, "T": T, "IA": IA, "IB": IB,
        "idxA": idxA, "idxB": idxB,
        "orig2pos": orig2pos, "pos2orig": pos2orig, "stats": stats,
    }


# ---------------------------------------------------------------------------
# Bass program
# ---------------------------------------------------------------------------

def build_program(cfg, topo, bias1_zero, bias2_zero):
    import concourse.bacc as bacc
    import concourse.mybir as mybir
    import concourse.tile as tile

    dt = mybir.dt
    S0, S1, T = topo["S0"], topo["S1"], topo["T"]
    IA, IB = topo["IA"], topo["IB"]
    NPAD, PERP, WIN, NB = cfg.NPAD, cfg.PERP, cfg.WIN, cfg.NB

    nc = bacc.Bacc("TRN2", target_bir_lowering=False, debug=False,
                   enable_asserts=False, num_devices=CORES,
                   num_swdge_queues=NQUEUES,
                   dynamic_dma_scratch_size=SCRATCH)

    # --- kernel I/O ---
    xTo_d = nc.dram_tensor("xTo", [F, PERP], dt.bfloat16, kind="ExternalInput")
    W1e_d = nc.dram_tensor("W1e", [F, 130], dt.bfloat16, kind="ExternalInput")
    W2e_d = nc.dram_tensor("W2e", [F, 130], dt.bfloat16, kind="ExternalInput")
    idxA_d = nc.dram_tensor("idxA", [128, max(IA, 16)], dt.int16, kind="ExternalInput")
    idxB_d = nc.dram_tensor("idxB", [128, max(IB, 16)], dt.int16, kind="ExternalInput")
    b1r_d = nc.dram_tensor("b1r", [128, F], dt.float32, kind="ExternalInput")
    b2r_d = nc.dram_tensor("b2r", [128, F], dt.float32, kind="ExternalInput")
    eye_d = nc.dram_tensor("eye", [128, 128], dt.float32, kind="ExternalInput")
    poi_d = nc.dram_tensor("poi", [128, 1], dt.float32, kind="ExternalInput")
    out_d = nc.dram_tensor("out", [PERP, F], dt.float32, kind="ExternalOutput")

    # --- internal DRAM: per-layer row shards and AllGathered tables ---
    cc1_in = nc.dram_tensor("cc1_in", [PERP * RB], dt.bfloat16)
    cc2_in = nc.dram_tensor("cc2_in", [PERP * RB], dt.bfloat16)
    cc1_out = nc.dram_tensor("cc1_out", [CORES, PERP * RB], dt.bfloat16,
                             addr_space="Shared")
    cc2_out = nc.dram_tensor("cc2_out", [CORES, PERP * RB], dt.bfloat16,
                             addr_space="Shared")

    with tile.TileContext(nc) as tc, ExitStack() as ctx:
        P = ctx.enter_context(tc.tile_pool(name="persist", bufs=1))
        pp = ctx.enter_context(tc.tile_pool(name="pp", bufs=3, space="PSUM"))
        pq = ctx.enter_context(tc.tile_pool(name="pq", bufs=2, space="PSUM"))
        gp = ctx.enter_context(tc.tile_pool(name="gp", bufs=4))
        wp = ctx.enter_context(tc.tile_pool(name="wp", bufs=2))
        sp = ctx.enter_context(tc.tile_pool(name="sp", bufs=4))
        apool = ctx.enter_context(tc.tile_pool(name="ap", bufs=3))

        # persistent SBUF
        idxA_s = P.tile([128, max(IA, 16)], dt.int16)
        idxB_s = P.tile([128, max(IB, 16)], dt.int16)
        W1e_s = P.tile([F, 130], dt.bfloat16)
        W2e_s = P.tile([F, 130], dt.bfloat16)
        b1r_s = P.tile([128, F], dt.float32)
        b2r_s = P.tile([128, F], dt.float32)
        eye_s = P.tile([128, 128], dt.float32)
        poi_s = P.tile([128, 1], dt.float32)
        adst1 = P.tile([128, NB], dt.float32)
        adst2 = P.tile([128, NB], dt.float32)
        xTo_s = P.tile([F, PERP], dt.bfloat16)
        x2Tb = P.tile([F, PERP], dt.bfloat16)
        selfh1 = P.tile([128, NB, RB], dt.bfloat16)
        selfh2 = P.tile([128, NB, RB], dt.bfloat16)

        nc.sync.dma_start(idxA_s[:], idxA_d[:])
        nc.sync.dma_start(idxB_s[:], idxB_d[:])
        nc.sync.dma_start(W1e_s[:], W1e_d[:])
        nc.sync.dma_start(W2e_s[:], W2e_d[:])
        nc.sync.dma_start(b1r_s[:], b1r_d[:])
        nc.sync.dma_start(b2r_s[:], b2r_d[:])
        nc.sync.dma_start(eye_s[:], eye_d[:])
        nc.sync.dma_start(poi_s[:], poi_d[:])
        nc.sync.dma_start(xTo_s[:], xTo_d[:])

        cc1v = cc1_in[:].rearrange("(n k) -> n k", k=RB)
        cc2v = cc2_in[:].rearrange("(n k) -> n k", k=RB)
        tab1 = cc1_out[:].rearrange("c (n k) -> (c n) k", k=RB)
        tab2 = cc2_out[:].rearrange("c (n k) -> (c n) k", k=RB)

        def assemble_row(ps, selfh, b, adst_s, cc_v):
            """Pack h row [124 bf16 | 4 fp8 | a_src f32] from psum, stash
            a_dst, and write the 256B row block to the local shard. The last
            block's last partition is the poison row for padded gather slots:
            its a_src is forced to MASK_NEG so exp() zeroes those slots."""
            s8 = selfh[:].bitcast(dt.float8e4)    # [128, NB, 2*RB]
            s32 = selfh[:].bitcast(dt.float32)    # [128, NB, RB//2]
            nc.scalar.activation(selfh[:, b, 0:NBF], ps[:, 0:NBF],
                                 mybir.ActivationFunctionType.Copy)
            nc.scalar.activation(s8[:, b, 2 * NBF:2 * NBF + NF8],
                                 ps[:, NBF:NBF + NF8],
                                 mybir.ActivationFunctionType.Copy)
            nc.scalar.activation(s32[:, b, ACOL:ACOL + 1], ps[:, 128:129],
                                 mybir.ActivationFunctionType.Copy)
            nc.vector.tensor_copy(adst_s[:, b:b + 1], ps[:, 129:130])
            nc.sync.dma_start(cc_v[128 * b:128 * (b + 1), :], selfh[:, b, :])
            if b == NB - 1:
                # poison the DRAM copy only: gathers of padded slots read this
                # row and see a_src = MASK_NEG; the SBUF self-row stays clean
                # so the poison position's own softmax keeps den >= 1.
                ccf = cc_v.bitcast(dt.float32)      # [PERP, RB//2]
                nc.sync.dma_start(ccf[PERP - 1:PERP, ACOL:ACOL + 1],
                                  poi_s[127:128, 0:1])

        def ag(cc_in_t, cc_out_t):
            nc.gpsimd.collective_compute(
                "AllGather", mybir.AluOpType.bypass,
                replica_groups=[list(range(CORES))],
                ins=[cc_in_t[:].opt()], outs=[cc_out_t[:].opt()])

        def h_stage1():
            for b in range(NB):
                ps = pp.tile([128, 130], dt.float32, tag="psh")
                nc.tensor.matmul(ps[:], xTo_s[:, 128 * b:128 * (b + 1)],
                                 W1e_s[:])
                assemble_row(ps, selfh1, b, adst1, cc1v)

        def edge_stage(layer, tab, selfh, adst_s, br_s, bias_zero):
            offs = []
            aoff = boff = 0
            for b in range(NB):
                offs.append((aoff, boff))
                aoff += 8 * int(S0[b])
                boff += 8 * int(S1[b])
            qn = [0]
            Gt = {}
            st = {}

            def stage_gather(b):
                s0, s1, t = int(S0[b]), int(S1[b]), int(T[b])
                ao, bo = offs[b]
                G = gp.tile([128, t, RB], dt.bfloat16, tag="G")
                Gt[b] = G
                nc.vector.tensor_copy(G[:, 0, :], selfh[:, b, :])
                for c0 in range(0, s0, GCHUNK):
                    cn = min(GCHUNK, s0 - c0)
                    nc.gpsimd.dma_gather(
                        G[:, 1 + c0:1 + c0 + cn, :], tab[0:WIN, :],
                        idxA_s[:, ao + 8 * c0:ao + 8 * (c0 + cn)],
                        128 * cn, 128 * cn, RB, queue_num=qn[0] % NQUEUES,
                        single_packet=SINGLE_PACKET)
                    qn[0] += 1
                for c0 in range(0, s1, GCHUNK):
                    cn = min(GCHUNK, s1 - c0)
                    nc.gpsimd.dma_gather(
                        G[:, 1 + s0 + c0:1 + s0 + c0 + cn, :],
                        tab[WIN:NPAD, :],
                        idxB_s[:, bo + 8 * c0:bo + 8 * (c0 + cn)],
                        128 * cn, 128 * cn, RB, queue_num=qn[0] % NQUEUES,
                        single_packet=SINGLE_PACKET)
                    qn[0] += 1

            def stage_front(b):
                t = int(T[b])
                G = Gt[b]
                G32 = G[:].bitcast(dt.float32)    # [128, t, RB//2]
                EL = sp.tile([128, t], dt.float32, tag="EL")
                if ACT_EL:
                    nc.scalar.activation(EL[:], G32[:, :, ACOL],
                                         mybir.ActivationFunctionType.Prelu,
                                         bias=adst_s[:, b:b + 1],
                                         alpha=NEG_SLOPE)
                else:
                    E = sp.tile([128, t], dt.float32, tag="E")
                    nc.vector.tensor_scalar_add(E[:], G32[:, :, ACOL],
                                                adst_s[:, b:b + 1])
                    nc.vector.scalar_tensor_tensor(
                        EL[:], E[:], NEG_SLOPE, E[:],
                        mybir.AluOpType.mult, mybir.AluOpType.max)
                EXb = sp.tile([128, t], dt.bfloat16, tag="EXb")
                den = sp.tile([128, 1], dt.float32, tag="den")
                nc.scalar.activation(EXb[:], EL[:],
                                     mybir.ActivationFunctionType.Exp,
                                     accum_out=den[:])
                if EX8_MODE:
                    EX8 = sp.tile([128, t, 8], dt.bfloat16, tag="EX8")
                    nc.scalar.activation(
                        EX8[:], EL[:].unsqueeze(2).broadcast_to([128, t, 8]),
                        mybir.ActivationFunctionType.Exp)
                else:
                    EX8 = None
                rec = sp.tile([128, 1], dt.float32, tag="rec")
                nc.vector.reciprocal(rec[:], den[:])
                st[b] = (EXb, EX8, rec)

            def stage_back(b):
                s0, s1, t = int(S0[b]), int(S1[b]), int(T[b])
                G = Gt.pop(b)
                EXb, EX8, rec = st.pop(b)
                G8 = G[:].bitcast(dt.float8e4)    # [128, t, 2*RB]
                wG = wp.tile([128, t, RB], dt.bfloat16, tag="wG")
                if EX8_MODE:
                    nc.vector.tensor_tensor(
                        wG[:].rearrange("p t (c e) -> p t c e", e=8),
                        G[:].rearrange("p t (c e) -> p t c e", e=8),
                        EX8[:].unsqueeze(2).broadcast_to([128, t, 16, 8]),
                        mybir.AluOpType.mult)
                else:
                    nc.vector.tensor_tensor(
                        wG[:, :, 0:NBF], G[:, :, 0:NBF],
                        EXb[:].unsqueeze(2).broadcast_to([128, t, NBF]),
                        mybir.AluOpType.mult)
                FD = sp.tile([128, t, NF8], dt.bfloat16, tag="FD")
                nc.vector.tensor_copy(FD[:], G8[:, :, 2 * NBF:2 * NBF + NF8])
                wFD = sp.tile([128, t, NF8], dt.bfloat16, tag="wFD")
                nc.vector.tensor_tensor(
                    wFD[:], FD[:],
                    EXb[:].unsqueeze(2).broadcast_to([128, t, NF8]),
                    mybir.AluOpType.mult)
                agg = apool.tile([128, F], dt.float32, tag="agg")
                cur = t
                for _ in range(TREE_LEVELS):
                    if cur > 2:
                        k = cur // 2
                        if EX8_MODE:
                            nc.vector.tensor_tensor(
                                wG[:, 0:k, :], wG[:, 0:k, :],
                                wG[:, cur - k:cur, :], mybir.AluOpType.add)
                        else:
                            nc.vector.tensor_tensor(
                                wG[:, 0:k, 0:NBF], wG[:, 0:k, 0:NBF],
                                wG[:, cur - k:cur, 0:NBF],
                                mybir.AluOpType.add)
                        cur -= k
                if cur == 1:
                    nc.vector.tensor_copy(agg[:, 0:NBF], wG[:, 0, 0:NBF])
                elif cur == 2:
                    nc.vector.tensor_tensor(agg[:, 0:NBF], wG[:, 0, 0:NBF],
                                            wG[:, 1, 0:NBF],
                                            mybir.AluOpType.add)
                else:
                    nc.vector.tensor_reduce(
                        agg[:, 0:NBF],
                        wG[:, 0:cur, 0:NBF].transpose([0, 2, 1]),
                        mybir.AxisListType.X, mybir.AluOpType.add)
                nc.vector.tensor_reduce(
                    agg[:, NBF:F], wFD[:].transpose([0, 2, 1]),
                    mybir.AxisListType.X, mybir.AluOpType.add)
                o2 = apool.tile([128, F], dt.float32, tag="o2")
                if bias_zero and ACT_OUT:
                    nc.scalar.activation(o2[:], agg[:],
                                         mybir.ActivationFunctionType.Relu,
                                         scale=rec[:, 0:1])
                else:
                    o = apool.tile([128, F], dt.float32, tag="o")
                    nc.vector.scalar_tensor_tensor(
                        o[:], agg[:], rec[:, 0:1], br_s[:],
                        mybir.AluOpType.mult, mybir.AluOpType.add)
                    nc.scalar.activation(o2[:], o[:],
                                         mybir.ActivationFunctionType.Relu)
                if layer == 1:
                    cols = slice(128 * b, 128 * (b + 1))
                    psT = pq.tile([128, 128], dt.float32, tag="psT")
                    nc.tensor.transpose(psT[:], o2[:], eye_s[:])
                    nc.scalar.activation(x2Tb[:, cols], psT[:],
                                         mybir.ActivationFunctionType.Copy)
                    ps2 = pp.tile([128, 130], dt.float32, tag="psh2")
                    nc.tensor.matmul(ps2[:], x2Tb[:, cols], W2e_s[:])
                    assemble_row(ps2, selfh2, b, adst2, cc2v)
                else:
                    nc.sync.dma_start(out_d[128 * b:128 * (b + 1), :], o2[:])

            if PIPE:
                for b in range(NB):
                    stage_gather(b)
                    if b >= 1:
                        stage_front(b - 1)
                    if b >= 2:
                        stage_back(b - 2)
                stage_front(NB - 1)
                stage_back(NB - 2)
                stage_back(NB - 1)
            else:
                for b in range(NB):
                    stage_gather(b)
                    stage_front(b)
                    stage_back(b)

        # ---- program ----
        h_stage1()
        ag(cc1_in, cc1_out)
        edge_stage(1, tab1, selfh1, adst1, b1r_s, bias1_zero)
        ag(cc2_in, cc2_out)
        edge_stage(2, tab2, selfh2, adst2, b2r_s, bias2_zero)

    nc.compile()
    return nc


# ---------------------------------------------------------------------------
# Host orchestration
# ---------------------------------------------------------------------------

def make_inputs(cfg, topo, x, W1, as1, ad1, b1, W2, as2, ad2, b2):
    PERP = cfg.PERP
    bf16 = ml_dtypes.bfloat16
    pos2orig = topo["pos2orig"]

    def wext(W, a_s, a_d):
        W = np.asarray(W, np.float64)
        return np.concatenate(
            [W, (W @ np.asarray(a_s, np.float64))[:, None],
             (W @ np.asarray(a_d, np.float64))[:, None]], axis=1
        ).astype(bf16)

    W1e = wext(W1, as1, ad1)
    W2e = wext(W2, as2, ad2)
    b1r = np.tile(np.asarray(b1, np.float32)[None, :], (128, 1))
    b2r = np.tile(np.asarray(b2, np.float32)[None, :], (128, 1))
    eye = np.eye(128, dtype=np.float32)
    poi = np.zeros((128, 1), np.float32)
    poi[127, 0] = MASK_NEG
    xf = np.asarray(x, np.float32)

    in_maps = []
    for k in range(CORES):
        po = pos2orig[PERP * k:PERP * (k + 1)]
        m = po >= 0
        xo = np.zeros((PERP, F), np.float32)
        xo[m] = xf[po[m]]
        in_maps.append({
            "xTo": np.ascontiguousarray(xo.T.astype(bf16)),
            "W1e": W1e, "W2e": W2e,
            "idxA": topo["idxA"][k],
            "idxB": topo["idxB"][k],
            "b1r": b1r, "b2r": b2r, "eye": eye, "poi": poi,
        })
    return in_maps


_CACHE = {}


def _get_program(cfg, edge_index, bias1_zero=True, bias2_zero=True):
    key = (hashlib.sha1(np.ascontiguousarray(edge_index).tobytes()).hexdigest(),
           bias1_zero, bias2_zero)
    if key not in _CACHE:
        topo = build_topology(cfg, edge_index)
        nc = build_program(cfg, topo, bias1_zero, bias2_zero)
        _CACHE[key] = (topo, nc)
    return _CACHE[key]


def run(cfg, inputs, trace=False):
    from concourse.bass_utils import run_bass_kernel_spmd

    b1z = not np.any(np.asarray(inputs["b1"]))
    b2z = not np.any(np.asarray(inputs["b2"]))
    topo, nc = _get_program(cfg, inputs["edge_index"], b1z, b2z)
    in_maps = make_inputs(
        cfg, topo, inputs["x"],
        inputs["W1"], inputs["att_src1"], inputs["att_dst1"], inputs["b1"],
        inputs["W2"], inputs["att_src2"], inputs["att_dst2"], inputs["b2"])
    res = run_bass_kernel_spmd(nc, in_maps, list(range(CORES)), trace=trace)

    full = np.zeros((cfg.N, F), np.float32)
    pos2orig = topo["pos2orig"]
    for k in range(CORES):
        o = np.asarray(res.results[k]["out"], np.float32)
        po = pos2orig[cfg.PERP * k:cfg.PERP * (k + 1)]
        m = po >= 0
        full[po[m]] = o[m]
    return full, res


def kernel(**inputs) -> np.ndarray:
    out, _ = run(FULL_CFG, inputs)
    return out


# revision 40
# speedup vs baseline: 1.0929x; 1.0929x over previous
"""Self-contained Trainium2 Bass kernel for a 2-layer GAT (GATConv x2, PyG-style).

Contract: kernel(**inputs) takes the FULL inputs (x [N,128] f32, edge_index
[2,E] int, W1/att_src1/att_dst1/b1/W2/att_src2/att_dst2/b2) and returns the
FULL [N,128] f32 output, distributing work across 8 NeuronCores internally.

v2 design (destination-sharded graph parallel):
  - Nodes are split by orig id into two halves; half h is dealt across cores
    4h..4h+3 by (w0,w1)-sorted rank so all 8 cores see near-identical degree
    sequences per 128-dst block (small slot padding).
  - Per layer each core computes h = x @ W_ext only for its OWN 6272 nodes
    (49 x 128), packs 256-byte rows [124 bf16 feats | 4 fp8 feats | a_src f32]
    and one AllGather builds the full 50176-row gather table on every core.
  - Per dst block, source rows are fetched with dma_gather (int16 indices,
    two 25088-row windows) into [128 dst, t, 128] bf16 tiles; softmax runs
    per partition (no max-subtraction needed: |e| is small, exp is safe in
    f32); weighted sum = 1x multiply + bf16 pair-tree + small f32 reduce.
  - Layer-2 h rows are produced inline as layer-1 edge blocks complete, so
    only the AllGather itself sits between the two edge phases.
"""

import hashlib
from contextlib import ExitStack

import ml_dtypes
import numpy as np

# ---------------------------------------------------------------------------
# Configuration
# ---------------------------------------------------------------------------

F = 128              # feature dim (all of F_in, H, F_out)
CORES = 8
RB = 128             # bf16 elements per table row (256 B)
NBF = 124            # leading bf16 feature columns in a row
NF8 = 4              # fp8 feature columns (features 124..127)
ACOL = 63            # f32 column (within bitcast row) holding a_src
NEG_SLOPE = 0.2
MASK_NEG = -30000.0

import os as _os
NQUEUES = int(_os.environ.get("GAT_NQUEUES", "4"))
GCHUNK = int(_os.environ.get("GAT_GCHUNK", "8"))      # slot-cols per dma_gather
SCRATCH = int(_os.environ.get("GAT_SCRATCH", "16384"))
SINGLE_PACKET = _os.environ.get("GAT_SINGLE_PACKET", "1") == "1"
TREE_LEVELS = int(_os.environ.get("GAT_TREE_LEVELS", "2"))
PIPE = _os.environ.get("GAT_PIPE", "0") == "1"
ACT_EL = _os.environ.get("GAT_ACT_EL", "1") == "1"
ACT_OUT = _os.environ.get("GAT_ACT_OUT", "1") == "1"
EX8_MODE = _os.environ.get("GAT_EX8", "0") == "1"


class Cfg:
    def __init__(self, n_nodes, per_core_blocks):
        self.N = n_nodes
        self.NB = per_core_blocks              # blocks of 128 dsts per core
        self.PERP = per_core_blocks * 128      # padded positions per core
        self.PER = n_nodes // CORES            # real nodes per core
        assert self.PER * CORES == n_nodes
        assert self.PERP >= self.PER
        self.NPAD = self.PERP * CORES
        self.WIN = self.PERP * (CORES // 2)    # gather window boundary
        assert self.WIN - 1 < 32768, "window must fit int16"


FULL_CFG = Cfg(50000, 49)


# ---------------------------------------------------------------------------
# Host-side topology preprocessing (pure graph structure, no feature math)
# ---------------------------------------------------------------------------

def _snake_order(w0, w1):
    """Order dsts by (w0, w1) with alternating w1 direction per w0-run, so
    consecutive groups have near-constant (w0, w1)."""
    idx = np.lexsort((w1, w0))
    w0s = w0[idx]
    out = []
    i = 0
    flip = False
    while i < len(idx):
        j = i
        while j < len(idx) and w0s[j] == w0s[i]:
            j += 1
        run = idx[i:j]
        out.append(run[::-1] if flip else run)
        flip = not flip
        i = j
    return np.concatenate(out)


def build_topology(cfg, edge_index):
    """Returns the shared block structure + per-core gather metadata."""
    src = np.asarray(edge_index[0], dtype=np.int64)
    dst = np.asarray(edge_index[1], dtype=np.int64)
    N, PER, PERP, NB = cfg.N, cfg.PER, cfg.PERP, cfg.NB
    half = N // 2  # orig-id window boundary (cores 0-3 vs 4-7)

    w0all = np.bincount(dst[src < half], minlength=N)
    w1all = np.bincount(dst[src >= half], minlength=N)

    # deal each half's (w0,w1)-sorted ranks across its 4 cores so per-block
    # degree sequences line up across cores
    orig2pos = np.full(N, -1, dtype=np.int64)
    S0k = np.zeros((CORES, NB), np.int64)
    S1k = np.zeros((CORES, NB), np.int64)
    for g in (0, 1):
        ids = np.arange(half * g, half * (g + 1))
        order = _snake_order(w0all[ids], w1all[ids])
        nodes = ids[order]                      # rank -> node id
        r = np.arange(len(nodes))
        core = 4 * g + (r % 4)
        pos = r // 4
        orig2pos[nodes] = PERP * core + pos
        for c in range(4):
            k = 4 * g + c
            w0s = w0all[nodes[c::4]]
            w1s = w1all[nodes[c::4]]
            w0p = np.concatenate([w0s, np.zeros(PERP - len(w0s), np.int64)])
            w1p = np.concatenate([w1s, np.zeros(PERP - len(w1s), np.int64)])
            S0k[k] = w0p.reshape(NB, 128).max(1)
            S1k[k] = w1p.reshape(NB, 128).max(1)

    S0 = S0k.max(0)
    S1 = S1k.max(0)
    T = 1 + S0 + S1                             # self col + both windows

    pos2orig = np.full(cfg.NPAD, -1, dtype=np.int64)
    valid = orig2pos >= 0
    pos2orig[orig2pos[valid]] = np.nonzero(valid)[0]

    IA = int(8 * S0.sum())
    IB = int(8 * S1.sum())
    idxA = np.zeros((CORES, 128, max(IA, 16)), np.int16)
    idxB = np.zeros((CORES, 128, max(IB, 16)), np.int16)

    # padded slots gather the "poison" rows (each core's last own position,
    # which is padding on every core); its a_src is set to MASK_NEG by the
    # kernel so exp() kills the slot without a separate mask tensor.
    poisonA = int(_os.environ.get('GAT_POIA', cfg.WIN - 1))
    poisonB = int(_os.environ.get('GAT_POIB', cfg.NPAD - 1 - cfg.WIN))

    dpos = orig2pos[dst]
    spos = orig2pos[src]
    dcore = dpos // PERP
    for k in range(CORES):
        m = dcore == k
        pos_of_dst = dpos[m] - PERP * k
        sp = spos[m]
        isw0 = sp < cfg.WIN
        bucket0 = [[] for _ in range(PERP)]
        bucket1 = [[] for _ in range(PERP)]
        for p, s_p, w in zip(pos_of_dst, sp, isw0):
            (bucket0 if w else bucket1)[p].append(s_p)
        aoff = boff = 0
        for b in range(NB):
            s0, s1 = int(S0[b]), int(S1[b])
            flatA = np.full(128 * s0, poisonA, np.int64)
            flatB = np.full(128 * s1, poisonB, np.int64)
            for p in range(128):
                gidx = 128 * b + p
                for s_i, s_p in enumerate(bucket0[gidx]):
                    flatA[s_i * 128 + p] = s_p
                for s_i, s_p in enumerate(bucket1[gidx]):
                    flatB[s_i * 128 + p] = s_p - cfg.WIN
            # wrap indices into [128, 8*n] int16 (16-row layout, replicated)
            for nfl, arr, off, tgt in ((s0, flatA, aoff, idxA),
                                       (s1, flatB, boff, idxB)):
                if nfl == 0:
                    continue
                cols = 8 * nfl
                wrapped = arr.reshape(cols, 16).T.astype(np.int16)  # [16, cols]
                tgt[k, :, off:off + cols] = np.tile(wrapped, (8, 1))
            aoff += 8 * s0
            boff += 8 * s1

    stats = {
        "real_edges": int(len(src)) + N,
        "padded_edges": int(T.sum()) * 128 * CORES,
    }
    return {
        "S0": S0, "S1": S1, "T": T, "IA": IA, "IB": IB,
        "idxA": idxA, "idxB": idxB,
        "orig2pos": orig2pos, "pos2orig": pos2orig, "stats": stats,
    }


# ---------------------------------------------------------------------------
# Bass program
# ---------------------------------------------------------------------------

def build_program(cfg, topo, bias1_zero, bias2_zero):
    import concourse.bacc as bacc
    import concourse.mybir as mybir
    import concourse.tile as tile

    dt = mybir.dt
    S0, S1, T = topo["S0"], topo["S1"], topo["T"]
    IA, IB = topo["IA"], topo["IB"]
    NPAD, PERP, WIN, NB = cfg.NPAD, cfg.PERP, cfg.WIN, cfg.NB

    nc = bacc.Bacc("TRN2", target_bir_lowering=False, debug=False,
                   enable_asserts=False, num_devices=CORES,
                   num_swdge_queues=NQUEUES,
                   dynamic_dma_scratch_size=SCRATCH)

    # --- kernel I/O ---
    xTo_d = nc.dram_tensor("xTo", [F, PERP], dt.bfloat16, kind="ExternalInput")
    W1e_d = nc.dram_tensor("W1e", [F, 130], dt.bfloat16, kind="ExternalInput")
    W2e_d = nc.dram_tensor("W2e", [F, 130], dt.bfloat16, kind="ExternalInput")
    idxA_d = nc.dram_tensor("idxA", [128, max(IA, 16)], dt.int16, kind="ExternalInput")
    idxB_d = nc.dram_tensor("idxB", [128, max(IB, 16)], dt.int16, kind="ExternalInput")
    b1r_d = nc.dram_tensor("b1r", [128, F], dt.float32, kind="ExternalInput")
    b2r_d = nc.dram_tensor("b2r", [128, F], dt.float32, kind="ExternalInput")
    eye_d = nc.dram_tensor("eye", [128, 128], dt.float32, kind="ExternalInput")
    poi_d = nc.dram_tensor("poi", [128, 1], dt.float32, kind="ExternalInput")
    out_d = nc.dram_tensor("out", [PERP, F], dt.float32, kind="ExternalOutput")

    # --- internal DRAM: per-layer row shards and AllGathered tables ---
    cc1_in = nc.dram_tensor("cc1_in", [PERP * RB], dt.bfloat16)
    cc2_in = nc.dram_tensor("cc2_in", [PERP * RB], dt.bfloat16)
    cc1_out = nc.dram_tensor("cc1_out", [CORES, PERP * RB], dt.bfloat16,
                             addr_space="Shared")
    cc2_out = nc.dram_tensor("cc2_out", [CORES, PERP * RB], dt.bfloat16,
                             addr_space="Shared")

    with tile.TileContext(nc) as tc, ExitStack() as ctx:
        P = ctx.enter_context(tc.tile_pool(name="persist", bufs=1))
        pp = ctx.enter_context(tc.tile_pool(name="pp", bufs=3, space="PSUM"))
        pq = ctx.enter_context(tc.tile_pool(name="pq", bufs=2, space="PSUM"))
        gp = ctx.enter_context(tc.tile_pool(name="gp", bufs=4))
        wp = ctx.enter_context(tc.tile_pool(name="wp", bufs=2))
        sp = ctx.enter_context(tc.tile_pool(name="sp", bufs=4))
        apool = ctx.enter_context(tc.tile_pool(name="ap", bufs=3))

        # persistent SBUF
        idxA_s = P.tile([128, max(IA, 16)], dt.int16)
        idxB_s = P.tile([128, max(IB, 16)], dt.int16)
        W1e_s = P.tile([F, 130], dt.bfloat16)
        W2e_s = P.tile([F, 130], dt.bfloat16)
        b1r_s = P.tile([128, F], dt.float32)
        b2r_s = P.tile([128, F], dt.float32)
        eye_s = P.tile([128, 128], dt.float32)
        poi_s = P.tile([128, 1], dt.float32)
        adst1 = P.tile([128, NB], dt.float32)
        adst2 = P.tile([128, NB], dt.float32)
        xTo_s = P.tile([F, PERP], dt.bfloat16)
        x2Tb = P.tile([F, PERP], dt.bfloat16)
        selfh1 = P.tile([128, NB, RB], dt.bfloat16)
        selfh2 = P.tile([128, NB, RB], dt.bfloat16)

        nc.sync.dma_start(idxA_s[:], idxA_d[:])
        nc.sync.dma_start(idxB_s[:], idxB_d[:])
        nc.sync.dma_start(W1e_s[:], W1e_d[:])
        nc.sync.dma_start(W2e_s[:], W2e_d[:])
        nc.sync.dma_start(b1r_s[:], b1r_d[:])
        nc.sync.dma_start(b2r_s[:], b2r_d[:])
        nc.sync.dma_start(eye_s[:], eye_d[:])
        nc.sync.dma_start(poi_s[:], poi_d[:])
        nc.sync.dma_start(xTo_s[:], xTo_d[:])

        cc1v = cc1_in[:].rearrange("(n k) -> n k", k=RB)
        cc2v = cc2_in[:].rearrange("(n k) -> n k", k=RB)
        tab1 = cc1_out[:].rearrange("c (n k) -> (c n) k", k=RB)
        tab2 = cc2_out[:].rearrange("c (n k) -> (c n) k", k=RB)

        def assemble_row(ps, selfh, b, adst_s, cc_v):
            """Pack h row [124 bf16 | 4 fp8 | a_src f32] from psum, stash
            a_dst, and write the 256B row block to the local shard. The last
            block's last partition is the poison row for padded gather slots:
            its a_src is forced to MASK_NEG so exp() zeroes those slots."""
            s8 = selfh[:].bitcast(dt.float8e4)    # [128, NB, 2*RB]
            s32 = selfh[:].bitcast(dt.float32)    # [128, NB, RB//2]
            nc.scalar.activation(selfh[:, b, 0:NBF], ps[:, 0:NBF],
                                 mybir.ActivationFunctionType.Copy)
            nc.scalar.activation(s8[:, b, 2 * NBF:2 * NBF + NF8],
                                 ps[:, NBF:NBF + NF8],
                                 mybir.ActivationFunctionType.Copy)
            nc.scalar.activation(s32[:, b, ACOL:ACOL + 1], ps[:, 128:129],
                                 mybir.ActivationFunctionType.Copy)
            nc.vector.tensor_copy(adst_s[:, b:b + 1], ps[:, 129:130])
            nc.sync.dma_start(cc_v[128 * b:128 * (b + 1), :], selfh[:, b, :])
            if b == NB - 1:
                # poison the DRAM copy only: gathers of padded slots read this
                # row and see a_src = MASK_NEG; the SBUF self-row stays clean
                # so the poison position's own softmax keeps den >= 1.
                ccf = cc_v.bitcast(dt.float32)      # [PERP, RB//2]
                nc.sync.dma_start(ccf[PERP - 1:PERP, ACOL:ACOL + 1],
                                  poi_s[127:128, 0:1])

        def ag(cc_in_t, cc_out_t):
            nc.gpsimd.collective_compute(
                "AllGather", mybir.AluOpType.bypass,
                replica_groups=[list(range(CORES))],
                ins=[cc_in_t[:].opt()], outs=[cc_out_t[:].opt()])

        def h_stage1():
            for b in range(NB):
                ps = pp.tile([128, 130], dt.float32, tag="psh")
                nc.tensor.matmul(ps[:], xTo_s[:, 128 * b:128 * (b + 1)],
                                 W1e_s[:])
                assemble_row(ps, selfh1, b, adst1, cc1v)

        def edge_stage(layer, tab, selfh, adst_s, br_s, bias_zero):
            offs = []
            aoff = boff = 0
            for b in range(NB):
                offs.append((aoff, boff))
                aoff += 8 * int(S0[b])
                boff += 8 * int(S1[b])
            qn = [0]
            Gt = {}
            st = {}

            def stage_gather(b):
                s0, s1, t = int(S0[b]), int(S1[b]), int(T[b])
                ao, bo = offs[b]
                G = gp.tile([128, t, RB], dt.bfloat16, tag="G")
                Gt[b] = G
                nc.vector.tensor_copy(G[:, 0, :], selfh[:, b, :])
                for c0 in range(0, s0, GCHUNK):
                    cn = min(GCHUNK, s0 - c0)
                    nc.gpsimd.dma_gather(
                        G[:, 1 + c0:1 + c0 + cn, :], tab[0:WIN, :],
                        idxA_s[:, ao + 8 * c0:ao + 8 * (c0 + cn)],
                        128 * cn, 128 * cn, RB, queue_num=qn[0] % NQUEUES,
                        single_packet=SINGLE_PACKET)
                    qn[0] += 1
                for c0 in range(0, s1, GCHUNK):
                    cn = min(GCHUNK, s1 - c0)
                    nc.gpsimd.dma_gather(
                        G[:, 1 + s0 + c0:1 + s0 + c0 + cn, :],
                        tab[WIN:NPAD, :],
                        idxB_s[:, bo + 8 * c0:bo + 8 * (c0 + cn)],
                        128 * cn, 128 * cn, RB, queue_num=qn[0] % NQUEUES,
                        single_packet=SINGLE_PACKET)
                    qn[0] += 1

            def stage_front(b):
                t = int(T[b])
                G = Gt[b]
                G32 = G[:].bitcast(dt.float32)    # [128, t, RB//2]
                EL = sp.tile([128, t], dt.float32, tag="EL")
                if ACT_EL:
                    nc.scalar.activation(EL[:], G32[:, :, ACOL],
                                         mybir.ActivationFunctionType.Prelu,
                                         bias=adst_s[:, b:b + 1],
                                         alpha=NEG_SLOPE)
                else:
                    E = sp.tile([128, t], dt.float32, tag="E")
                    nc.vector.tensor_scalar_add(E[:], G32[:, :, ACOL],
                                                adst_s[:, b:b + 1])
                    nc.vector.scalar_tensor_tensor(
                        EL[:], E[:], NEG_SLOPE, E[:],
                        mybir.AluOpType.mult, mybir.AluOpType.max)
                EXb = sp.tile([128, t], dt.bfloat16, tag="EXb")
                den = sp.tile([128, 1], dt.float32, tag="den")
                nc.scalar.activation(EXb[:], EL[:],
                                     mybir.ActivationFunctionType.Exp,
                                     accum_out=den[:])
                if EX8_MODE:
                    EX8 = sp.tile([128, t, 8], dt.bfloat16, tag="EX8")
                    nc.scalar.activation(
                        EX8[:], EL[:].unsqueeze(2).broadcast_to([128, t, 8]),
                        mybir.ActivationFunctionType.Exp)
                else:
                    EX8 = None
                rec = sp.tile([128, 1], dt.float32, tag="rec")
                nc.vector.reciprocal(rec[:], den[:])
                st[b] = (EXb, EX8, rec)

            def stage_back(b):
                s0, s1, t = int(S0[b]), int(S1[b]), int(T[b])
                G = Gt.pop(b)
                EXb, EX8, rec = st.pop(b)
                G8 = G[:].bitcast(dt.float8e4)    # [128, t, 2*RB]
                wG = wp.tile([128, t, RB], dt.bfloat16, tag="wG")
                if EX8_MODE:
                    nc.vector.tensor_tensor(
                        wG[:].rearrange("p t (c e) -> p t c e", e=8),
                        G[:].rearrange("p t (c e) -> p t c e", e=8),
                        EX8[:].unsqueeze(2).broadcast_to([128, t, 16, 8]),
                        mybir.AluOpType.mult)
                else:
                    nc.vector.tensor_tensor(
                        wG[:, :, 0:NBF], G[:, :, 0:NBF],
                        EXb[:].unsqueeze(2).broadcast_to([128, t, NBF]),
                        mybir.AluOpType.mult)
                FD = sp.tile([128, t, NF8], dt.bfloat16, tag="FD")
                nc.vector.tensor_copy(FD[:], G8[:, :, 2 * NBF:2 * NBF + NF8])
                wFD = sp.tile([128, t, NF8], dt.bfloat16, tag="wFD")
                nc.vector.tensor_tensor(
                    wFD[:], FD[:],
                    EXb[:].unsqueeze(2).broadcast_to([128, t, NF8]),
                    mybir.AluOpType.mult)
                agg = apool.tile([128, F], dt.float32, tag="agg")
                cur = t
                for _ in range(TREE_LEVELS):
                    if cur > 2:
                        k = cur // 2
                        if EX8_MODE:
                            nc.vector.tensor_tensor(
                                wG[:, 0:k, :], wG[:, 0:k, :],
                                wG[:, cur - k:cur, :], mybir.AluOpType.add)
                        else:
                            nc.vector.tensor_tensor(
                                wG[:, 0:k, 0:NBF], wG[:, 0:k, 0:NBF],
                                wG[:, cur - k:cur, 0:NBF],
                                mybir.AluOpType.add)
                        cur -= k
                if cur == 1:
                    nc.vector.tensor_copy(agg[:, 0:NBF], wG[:, 0, 0:NBF])
                elif cur == 2:
                    nc.vector.tensor_tensor(agg[:, 0:NBF], wG[:, 0, 0:NBF],
                                            wG[:, 1, 0:NBF],
                                            mybir.AluOpType.add)
                else:
                    nc.vector.tensor_reduce(
                        agg[:, 0:NBF],
                        wG[:, 0:cur, 0:NBF].transpose([0, 2, 1]),
                        mybir.AxisListType.X, mybir.AluOpType.add)
                nc.vector.tensor_reduce(
                    agg[:, NBF:F], wFD[:].transpose([0, 2, 1]),
                    mybir.AxisListType.X, mybir.AluOpType.add)
                o2 = apool.tile([128, F], dt.float32, tag="o2")
                if bias_zero and ACT_OUT:
                    nc.scalar.activation(o2[:], agg[:],
                                         mybir.ActivationFunctionType.Relu,
                                         scale=rec[:, 0:1])
                else:
                    o = apool.tile([128, F], dt.float32, tag="o")
                    nc.vector.scalar_tensor_tensor(
                        o[:], agg[:], rec[:, 0:1], br_s[:],
                        mybir.AluOpType.mult, mybir.AluOpType.add)
                    nc.scalar.activation(o2[:], o[:],
                                         mybir.ActivationFunctionType.Relu)
                if layer == 1:
                    cols = slice(128 * b, 128 * (b + 1))
                    psT = pq.tile([128, 128], dt.float32, tag="psT")
                    nc.tensor.transpose(psT[:], o2[:], eye_s[:])
                    nc.scalar.activation(x2Tb[:, cols], psT[:],
                                         mybir.ActivationFunctionType.Copy)
                    ps2 = pp.tile([128, 130], dt.float32, tag="psh2")
                    nc.tensor.matmul(ps2[:], x2Tb[:, cols], W2e_s[:])
                    assemble_row(ps2, selfh2, b, adst2, cc2v)
                else:
                    nc.sync.dma_start(out_d[128 * b:128 * (b + 1), :], o2[:])

            if PIPE:
                for b in range(NB):
                    stage_gather(b)
                    if b >= 1:
                        stage_front(b - 1)
                    if b >= 2:
                        stage_back(b - 2)
                stage_front(NB - 1)
                stage_back(NB - 2)
                stage_back(NB - 1)
            else:
                for b in range(NB):
                    stage_gather(b)
                    stage_front(b)
                    stage_back(b)

        # ---- program ----
        h_stage1()
        ag(cc1_in, cc1_out)
        edge_stage(1, tab1, selfh1, adst1, b1r_s, bias1_zero)
        ag(cc2_in, cc2_out)
        edge_stage(2, tab2, selfh2, adst2, b2r_s, bias2_zero)

    nc.compile()
    return nc


# ---------------------------------------------------------------------------
# Host orchestration
# ---------------------------------------------------------------------------

def make_inputs(cfg, topo, x, W1, as1, ad1, b1, W2, as2, ad2, b2):
    PERP = cfg.PERP
    bf16 = ml_dtypes.bfloat16
    pos2orig = topo["pos2orig"]

    def wext(W, a_s, a_d):
        W = np.asarray(W, np.float64)
        return np.concatenate(
            [W, (W @ np.asarray(a_s, np.float64))[:, None],
             (W @ np.asarray(a_d, np.float64))[:, None]], axis=1
        ).astype(bf16)

    W1e = wext(W1, as1, ad1)
    W2e = wext(W2, as2, ad2)
    b1r = np.tile(np.asarray(b1, np.float32)[None, :], (128, 1))
    b2r = np.tile(np.asarray(b2, np.float32)[None, :], (128, 1))
    eye = np.eye(128, dtype=np.float32)
    poi = np.zeros((128, 1), np.float32)
    poi[127, 0] = MASK_NEG
    xf = np.asarray(x, np.float32)

    in_maps = []
    for k in range(CORES):
        po = pos2orig[PERP * k:PERP * (k + 1)]
        m = po >= 0
        xo = np.zeros((PERP, F), np.float32)
        xo[m] = xf[po[m]]
        in_maps.append({
            "xTo": np.ascontiguousarray(xo.T.astype(bf16)),
            "W1e": W1e, "W2e": W2e,
            "idxA": topo["idxA"][k],
            "idxB": topo["idxB"][k],
            "b1r": b1r, "b2r": b2r, "eye": eye, "poi": poi,
        })
    return in_maps


_CACHE = {}


def _get_program(cfg, edge_index, bias1_zero=True, bias2_zero=True):
    key = (hashlib.sha1(np.ascontiguousarray(edge_index).tobytes()).hexdigest(),
           bias1_zero, bias2_zero)
    if key not in _CACHE:
        topo = build_topology(cfg, edge_index)
        nc = build_program(cfg, topo, bias1_zero, bias2_zero)
        _CACHE[key] = (topo, nc)
    return _CACHE[key]


def run(cfg, inputs, trace=False):
    from concourse.bass_utils import run_bass_kernel_spmd

    b1z = not np.any(np.asarray(inputs["b1"]))
    b2z = not np.any(np.asarray(inputs["b2"]))
    topo, nc = _get_program(cfg, inputs["edge_index"], b1z, b2z)
    in_maps = make_inputs(
        cfg, topo, inputs["x"],
        inputs["W1"], inputs["att_src1"], inputs["att_dst1"], inputs["b1"],
        inputs["W2"], inputs["att_src2"], inputs["att_dst2"], inputs["b2"])
    res = run_bass_kernel_spmd(nc, in_maps, list(range(CORES)), trace=trace)

    full = np.zeros((cfg.N, F), np.float32)
    pos2orig = topo["pos2orig"]
    for k in range(CORES):
        o = np.asarray(res.results[k]["out"], np.float32)
        po = pos2orig[cfg.PERP * k:cfg.PERP * (k + 1)]
        m = po >= 0
        full[po[m]] = o[m]
    return full, res


def kernel(**inputs) -> np.ndarray:
    out, _ = run(FULL_CFG, inputs)
    return out
